# revision 7
# baseline (speedup 1.0000x reference)
"""MoE layer (8 experts, top-2, SwiGLU) for Trainium2, expert-parallel over 8 cores.

Strategy:
  - Router (x @ router_w, top-2, softmax) runs on host in fp32 — it is 0.01%
    of the FLOPs and determines the (data-dependent) sharding.
  - Each core is assigned one expert. Tokens routed to that expert are
    gathered on host, padded to a common capacity C, and shipped transposed
    as xT [D, C] so both GEMMs need no on-device transpose:
        h1T = w1.T @ x.T   (lhsT = w1 [D,Hp], rhs = xT [D,C])   -> [Hp, C]
        h2T = w2.T @ x.T
        hT  = silu(h1T) * h2T
        y   = hT.T @ w3    (lhsT = hT [Hp,C], rhs = w3 [Hp,D])  -> [C, D]
    y rows are scaled by the per-token combine weight on device.
  - Host scatter-adds the 8 per-expert outputs back to [B,S,D].

  Matmuls run in bf16 (fp32 accumulate in PSUM); hidden dim 2730 is padded
  to 2816 = 22*128 (zero pad is exact: silu(0)*0 = 0).

  HW A/B (microbench.py) showed PSUM bank sequencing dominates PE
  throughput on this silicon: ALL matmul outputs rotate through one PSUM
  pool/tag ("psZ"); accumulation runs per bank region are kept contiguous
  (interleaving two accumulation groups is numerically WRONG on hw), and
  runs deeper than ~8 MMs into one region measure slower (~265 ns/MM for
  22-deep vs ~180-230 for <=8-deep bursts at N=512), so mm3's 22-chunk
  contraction is split into 3 partials of depth 8/8/6 that DVE folds into
  the output as (p*wv)+acc via scalar_tensor_tensor.
  y is stored bf16 (host accumulates in fp32; adds ~2e-4 rel err) and
  leaves on the ACT HWDGE queue to stay off the x/w load queue.

  Measured (rep-slope, interleaved 1/128-rep pairing): ~607-623 us vs
  TimelineSim 504 us — consistent with the P0 power downclock to ~2.0 GHz
  under sustained 8-core matmul load; fp8 (4.6e-2 err) and capacity drops
  (3.7e-2 err) both blow the 2e-2 budget, so bf16 at this schedule is the
  practical wall.
"""

import os

import numpy as np
import ml_dtypes

DIM = 1024
NUM_EXPERTS = 8
HIDDEN = 2730
P = 128
HP = 2816  # hidden padded to 22*128
KD = DIM // P  # 8 contraction chunks for mm1/mm2
HPT = HP // P  # 22 chunks of the hidden dim
NBLK = 512  # token block (moving free dim per matmul)

TRACE = os.environ.get("MOE_TRACE", "0") == "1"
PAIR = os.environ.get("MOE_PAIR", "0") == "1"
YBF16 = os.environ.get("MOE_YBF16", "1") == "1"
LAST_RESULT = None  # BassKernelResults of the last run (for test harness)

_KERNELS: dict = {}


def _build(C: int, c_real: int | None = None, reps: int = 1, pair: bool = PAIR):
    """Build + compile the per-core Bass kernel for capacity C (multiple of 128).

    reps > 1 wraps the whole body (including DMAs) in a device-side loop that
    recomputes the same result `reps` times — only used for wall-clock
    benchmarking (dispatch overhead cancels in the rep delta).

    All matmul outputs rotate through ONE 2-bank-tile PSUM pool/tag (HW A/B:
    single-tag rotation ~147 ns/MM vs ~250 for multi-tag alternation), and
    consumers are batched two hidden-chunks (mm1/mm2) or two output halves
    (mm3) per instruction to halve ACT/DVE PSUM-read op count.
    """
    import concourse.mybir as mybir
    import concourse.tile as tile
    from concourse import bacc

    dt = mybir.dt
    nc = bacc.Bacc(None, target_bir_lowering=False)

    xt = nc.dram_tensor("xt", [KD, P, C], dt.bfloat16, kind="ExternalInput")
    w1 = nc.dram_tensor("w1", [KD, P, HP], dt.bfloat16, kind="ExternalInput")
    w2 = nc.dram_tensor("w2", [KD, P, HP], dt.bfloat16, kind="ExternalInput")
    w3 = nc.dram_tensor("w3", [HPT, P, DIM], dt.bfloat16, kind="ExternalInput")
    wv = nc.dram_tensor("wv", [P, C // P], dt.float32, kind="ExternalInput")
    ydt = dt.bfloat16 if YBF16 else dt.float32
    y = nc.dram_tensor("y", [C, DIM], ydt, kind="ExternalOutput")

    # Only c_real tokens are real; rows beyond that are padding whose
    # output the host ignores, so the last block shrinks to the real count.
    if c_real is None:
        c_real = C
    blocks = []
    c0 = 0
    while c0 < c_real:
        bn = min(NBLK, c_real - c0)
        blocks.append((c0, bn))
        c0 += bn
    HP2 = HPT // 2  # hidden chunk pairs

    with tile.TileContext(nc) as tc:
        with (
            tc.tile_pool(name="wpool", bufs=1) as wpool,
            tc.tile_pool(name="xpool", bufs=2) as xpool,
            tc.tile_pool(name="hpool", bufs=1) as hpool,
            tc.tile_pool(name="tpool", bufs=2) as tpool,
            tc.tile_pool(name="apool", bufs=2) as apool,
            tc.tile_pool(name="ypool", bufs=3 if YBF16 else 2) as ypool,
            # 4 slots x [P, 1024] f32 = all 8 PSUM banks, one shared tag
            tc.tile_pool(name="psZ", bufs=4, space="PSUM") as psZ,
        ):

            def make_x(bi, c0, bn):
                t = xpool.tile(
                    [P, KD, NBLK], dt.bfloat16, name=f"x_{bi}", tag="x"
                )
                for kd in range(KD):
                    nc.sync.dma_start(t[:, kd, :bn], xt[kd][:, c0 : c0 + bn])
                return t

            def emit_body():
                # First block's activations, so mm1 can start early.
                x0 = make_x(0, blocks[0][0], blocks[0][1])

                # Resident weights, DMA'd in hp-sliced parts in the order the
                # first block's matmuls consume them, split across the SP HWDGE
                # queue (x, w1, w3) and the gpsimd SWDGE queue (w2, wv).
                w1_sb = [
                    wpool.tile([P, HP], dt.bfloat16, name=f"w1_{kd}", tag=f"w1_{kd}")
                    for kd in range(KD)
                ]
                w2_sb = [
                    wpool.tile([P, HP], dt.bfloat16, name=f"w2_{kd}", tag=f"w2_{kd}")
                    for kd in range(KD)
                ]
                w3_sb = [
                    wpool.tile([P, DIM], dt.bfloat16, name=f"w3_{hp}", tag=f"w3_{hp}")
                    for hp in range(HPT)
                ]
                bounds = [0, 3 * P, 7 * P, 12 * P, 17 * P, HP]
                for pi in range(len(bounds) - 1):
                    sl = slice(bounds[pi], bounds[pi + 1])
                    for kd in range(KD):
                        nc.sync.dma_start(w1_sb[kd][:, sl], w1[kd][:, sl])
                    for kd in range(KD):
                        nc.gpsimd.dma_start(w2_sb[kd][:, sl], w2[kd][:, sl])
                for hp in range(HPT):
                    nc.sync.dma_start(w3_sb[hp][:], w3[hp])

                wv_sb = wpool.tile([P, C // P], dt.float32, name="wv_sb", tag="wv_sb")
                nc.gpsimd.dma_start(wv_sb[:], wv[:])

                for bi, (c0, bn) in enumerate(blocks):
                    xb = x0 if bi == 0 else make_x(bi, c0, bn)

                    # hT = silu(w1.T @ xT) * (w2.T @ xT), two hidden chunks
                    # (2*128 rows) per PSUM tile / consumer op.
                    h_sbs = []
                    for hp2 in range(HP2):
                        # NOTE: kd-outer/half-inner interleaving of the two
                        # accumulation groups was tried for rhs reuse and is
                        # NUMERICALLY WRONG on hardware (rel err 4e-2) —
                        # accumulation runs per PSUM region must be contiguous.
                        ps1 = psZ.tile(
                            [P, 2 * bn], dt.float32,
                            name=f"ps1_{bi}_{hp2}", tag="ps",
                        )
                        for half in range(2):
                            hp = 2 * hp2 + half
                            osl = slice(half * bn, (half + 1) * bn)
                            for kd in range(KD):
                                nc.tensor.matmul(
                                    ps1[:, osl],
                                    w1_sb[kd][:, hp * P : (hp + 1) * P],
                                    xb[:, kd, 0:bn],
                                    start=(kd == 0),
                                    stop=(kd == KD - 1),
                                )
                        tsil = tpool.tile(
                            [P, 2 * bn], dt.float32,
                            name=f"sil_{bi}_{hp2}", tag="sil",
                        )
                        nc.scalar.activation(
                            tsil[:], ps1[:], mybir.ActivationFunctionType.Silu
                        )
                        ps2 = psZ.tile(
                            [P, 2 * bn], dt.float32,
                            name=f"ps2_{bi}_{hp2}", tag="ps",
                        )
                        for half in range(2):
                            hp = 2 * hp2 + half
                            osl = slice(half * bn, (half + 1) * bn)
                            for kd in range(KD):
                                nc.tensor.matmul(
                                    ps2[:, osl],
                                    w2_sb[kd][:, hp * P : (hp + 1) * P],
                                    xb[:, kd, 0:bn],
                                    start=(kd == 0),
                                    stop=(kd == KD - 1),
                                )
                        ht = hpool.tile(
                            [P, 2 * bn], dt.bfloat16,
                            name=f"h_{bi}_{hp2}", tag=f"h_{hp2}",
                        )
                        nc.vector.tensor_mul(ht[:], tsil[:], ps2[:])
                        h_sbs.append(ht)

                    # y[block] = (hT.T @ w3) * combine_weight. HW A/B: PSUM
                    # accumulation runs deeper than ~8 into one bank region
                    # degrade the sustained MM rate (~265 ns/MM for 22-deep
                    # vs ~180-200 for 8-deep bursts), so the 22-chunk
                    # contraction is split into 3 partials of depth 8/8/6 —
                    # each a fresh pair of <=8-long bank bursts — and DVE
                    # folds each partial into the output as (p*wv) + acc.
                    for cs in range((bn + P - 1) // P):
                        M = min(P, bn - cs * P)
                        ci = c0 // P + cs
                        acc = None
                        for pi, (lo_hp, hi_hp) in enumerate(
                            [(0, 8), (8, 16), (16, HPT)]
                        ):
                            ps3 = psZ.tile(
                                [P, DIM], dt.float32,
                                name=f"ps3_{bi}_{cs}_{pi}", tag="ps",
                            )
                            for dti in range(DIM // 512):
                                dsl = slice(dti * 512, (dti + 1) * 512)
                                for hp in range(lo_hp, hi_hp):
                                    hp2, half = divmod(hp, 2)
                                    lo = half * bn + cs * P
                                    nc.tensor.matmul(
                                        ps3[:M, dsl],
                                        h_sbs[hp2][:, lo : lo + M],
                                        w3_sb[hp][:, dsl],
                                        start=(hp == lo_hp),
                                        stop=(hp == hi_hp - 1),
                                    )
                            wvs = wv_sb[:M, ci : ci + 1]
                            if pi == 0:
                                a1 = apool.tile(
                                    [P, DIM], dt.float32,
                                    name=f"ya_{bi}_{cs}", tag="ya",
                                )
                                nc.vector.tensor_scalar_mul(a1[:M], ps3[:M], wvs)
                                acc = a1
                            elif pi == 1:
                                a2 = apool.tile(
                                    [P, DIM], dt.float32,
                                    name=f"yb_{bi}_{cs}", tag="yb",
                                )
                                nc.vector.scalar_tensor_tensor(
                                    a2[:M], ps3[:M], wvs, acc[:M],
                                    op0=mybir.AluOpType.mult,
                                    op1=mybir.AluOpType.add,
                                )
                                acc = a2
                            else:
                                yt = ypool.tile(
                                    [P, DIM], ydt, name=f"y_{bi}_{cs}", tag="yt",
                                )
                                nc.vector.scalar_tensor_tensor(
                                    yt[:M], ps3[:M], wvs, acc[:M],
                                    op0=mybir.AluOpType.mult,
                                    op1=mybir.AluOpType.add,
                                )
                        # y goes out on the (otherwise idle) ACT HWDGE queue
                        # so stores don't serialize behind x/w loads
                        nc.scalar.dma_start(
                            y[c0 + cs * P : c0 + cs * P + M, :], yt[:M]
                        )

            if reps > 1:
                # PE body is ~5.6k instructions (>> one 16 KiB IRAM block):
                # without a branch hint every back-edge stalls ~4 us on the
                # IRAM refetch of the loop-start block.
                with tc.For_i(0, reps, 1, hint_engines=(mybir.EngineType.PE,)):
                    emit_body()
            else:
                emit_body()

    nc.compile()
    return nc


def _route(xf: np.ndarray, router_w: np.ndarray):
    """Top-2 routing + softmax weights, fp32, matching the jax reference."""
    T = xf.shape[0]
    logits = xf @ router_w  # [T, E]
    rows = np.arange(T)
    i1 = logits.argmax(axis=1)
    tmp = logits.copy()
    tmp[rows, i1] = -np.inf
    i2 = tmp.argmax(axis=1)
    v1 = logits[rows, i1]
    v2 = tmp[rows, i2]
    e2 = np.exp((v2 - v1).astype(np.float32))
    g1 = 1.0 / (1.0 + e2)
    g2 = e2 / (1.0 + e2)
    return i1, i2, g1.astype(np.float32), g2.astype(np.float32)


def _prepare(x, router_w, w1, w2, w3):
    """Route + dispatch on host; returns (C, in_maps, idxs, shape)."""
    x = np.asarray(x, dtype=np.float32)
    router_w = np.asarray(router_w, dtype=np.float32)
    w1 = np.asarray(w1, dtype=np.float32)
    w2 = np.asarray(w2, dtype=np.float32)
    w3 = np.asarray(w3, dtype=np.float32)

    B, S, D = x.shape
    T = B * S
    xf = x.reshape(T, D)

    i1, i2, g1, g2 = _route(xf, router_w)

    # per-expert token lists (slot-0 tokens then slot-1 tokens)
    idxs, wgts = [], []
    for e in range(NUM_EXPERTS):
        s0 = np.nonzero(i1 == e)[0]
        s1 = np.nonzero(i2 == e)[0]
        idxs.append(np.concatenate([s0, s1]))
        wgts.append(np.concatenate([g1[s0], g2[s1]]))
    max_cnt = max(len(ix) for ix in idxs)
    C = max(P, ((max_cnt + P - 1) // P) * P)

    bf16 = ml_dtypes.bfloat16
    # expert weights, padded along the hidden dim and cast to bf16
    w1p = np.zeros((NUM_EXPERTS, D, HP), dtype=bf16)
    w1p[:, :, :HIDDEN] = w1
    w2p = np.zeros((NUM_EXPERTS, D, HP), dtype=bf16)
    w2p[:, :, :HIDDEN] = w2
    w3p = np.zeros((NUM_EXPERTS, HP, D), dtype=bf16)
    w3p[:, :HIDDEN, :] = w3

    in_maps = []
    for e in range(NUM_EXPERTS):
        ix = idxs[e]
        xg = np.zeros((C, D), dtype=np.float32)
        xg[: len(ix)] = xf[ix]
        wvec = np.zeros((C,), dtype=np.float32)
        wvec[: len(ix)] = wgts[e]
        wvec = np.ascontiguousarray(wvec.reshape(C // P, P).T)  # [P, C//P]
        in_maps.append(
            {
                "xt": np.ascontiguousarray(xg.T).astype(bf16).reshape(KD, P, C),
                "w1": w1p[e].reshape(KD, P, HP),
                "w2": w2p[e].reshape(KD, P, HP),
                "w3": w3p[e].reshape(HPT, P, DIM),
                "wv": wvec,
            }
        )
    return C, in_maps, idxs, (B, S, D)


def kernel(x, router_w, w1, w2, w3):
    global LAST_RESULT
    from concourse.bass_utils import run_bass_kernel_spmd

    C, in_maps, idxs, (B, S, D) = _prepare(x, router_w, w1, w2, w3)

    max_cnt = max(len(ix) for ix in idxs)
    key = (C, max_cnt)
    if key not in _KERNELS:
        _KERNELS[key] = _build(C, c_real=max_cnt)
    nc = _KERNELS[key]

    res = run_bass_kernel_spmd(
        nc,
        in_maps,
        list(range(NUM_EXPERTS)),
        trace=TRACE,
    )
    LAST_RESULT = res

    out = np.zeros((B * S, D), dtype=np.float32)
    for e in range(NUM_EXPERTS):
        ix = idxs[e]
        out[ix] += res.results[e]["y"][: len(ix)]
    return out.reshape(B, S, D)



# revision 10
# speedup vs baseline: 1.0220x; 1.0220x over previous
"""MoE layer (8 experts, top-2, SwiGLU) for Trainium2, expert-parallel over 8 cores.

Strategy:
  - Router (x @ router_w, top-2, softmax) runs on host in fp32 — it is 0.01%
    of the FLOPs and determines the (data-dependent) sharding.
  - Each core is assigned one expert. Tokens routed to that expert are
    gathered on host, padded to a common capacity C, and shipped transposed
    as xT [D, C] so both GEMMs need no on-device transpose:
        h1T = w1.T @ x.T   (lhsT = w1 [D,Hp], rhs = xT [D,C])   -> [Hp, C]
        h2T = w2.T @ x.T
        hT  = silu(h1T) * h2T
        y   = hT.T @ w3    (lhsT = hT [Hp,C], rhs = w3 [Hp,D])  -> [C, D]
    y rows are scaled by the per-token combine weight on device.
  - Host scatter-adds the 8 per-expert outputs back to [B,S,D].

  Matmuls run in bf16 (fp32 accumulate in PSUM); hidden dim 2730 is padded
  to 2816 = 22*128 (zero pad is exact: silu(0)*0 = 0).

  HW A/B (microbench.py) showed PSUM bank sequencing dominates PE
  throughput on this silicon: ALL matmul outputs rotate through one PSUM
  pool/tag ("psZ"); accumulation runs per bank region are kept contiguous
  (interleaving two accumulation groups is numerically WRONG on hw), and
  runs deeper than ~8 MMs into one region measure slower (~265 ns/MM for
  22-deep vs ~180-230 for <=8-deep bursts at N=512), so mm3's 22-chunk
  contraction is split into 3 partials of depth 8/8/6 that DVE folds into
  the output as (p*wv)+acc via scalar_tensor_tensor.
  y is stored bf16 (host accumulates in fp32; adds ~2e-4 rel err) and
  leaves on the ACT HWDGE queue to stay off the x/w load queue.

  Measured (rep-slope, interleaved 1/128-rep pairing): ~607-623 us vs
  TimelineSim 504 us — consistent with the P0 power downclock to ~2.0 GHz
  under sustained 8-core matmul load; fp8 (4.6e-2 err) and capacity drops
  (3.7e-2 err) both blow the 2e-2 budget, so bf16 at this schedule is the
  practical wall.
"""

import os

import numpy as np
import ml_dtypes

DIM = 1024
NUM_EXPERTS = 8
HIDDEN = 2730
P = 128
HP = 2816  # hidden padded to 22*128
KD = DIM // P  # 8 contraction chunks for mm1/mm2
HPT = HP // P  # 22 chunks of the hidden dim
NBLK = 512  # token block (moving free dim per matmul)

TRACE = os.environ.get("MOE_TRACE", "0") == "1"
PAIR = os.environ.get("MOE_PAIR", "0") == "1"
YBF16 = os.environ.get("MOE_YBF16", "1") == "1"
LAST_RESULT = None  # BassKernelResults of the last run (for test harness)

_KERNELS: dict = {}


def _build(C: int, c_real: int | None = None, reps: int = 1, pair: bool = PAIR):
    """Build + compile the per-core Bass kernel for capacity C (multiple of 128).

    reps > 1 wraps the whole body (including DMAs) in a device-side loop that
    recomputes the same result `reps` times — only used for wall-clock
    benchmarking (dispatch overhead cancels in the rep delta).

    All matmul outputs rotate through ONE 2-bank-tile PSUM pool/tag (HW A/B:
    single-tag rotation ~147 ns/MM vs ~250 for multi-tag alternation), and
    consumers are batched two hidden-chunks (mm1/mm2) or two output halves
    (mm3) per instruction to halve ACT/DVE PSUM-read op count.
    """
    import concourse.mybir as mybir
    import concourse.tile as tile
    from concourse import bacc

    dt = mybir.dt
    nc = bacc.Bacc(None, target_bir_lowering=False)

    xt = nc.dram_tensor("xt", [KD, P, C], dt.bfloat16, kind="ExternalInput")
    w1 = nc.dram_tensor("w1", [KD, P, HP], dt.bfloat16, kind="ExternalInput")
    w2 = nc.dram_tensor("w2", [KD, P, HP], dt.bfloat16, kind="ExternalInput")
    w3 = nc.dram_tensor("w3", [HPT, P, DIM], dt.bfloat16, kind="ExternalInput")
    wv = nc.dram_tensor("wv", [P, C // P], dt.float32, kind="ExternalInput")
    ydt = dt.bfloat16 if YBF16 else dt.float32
    y = nc.dram_tensor("y", [C, DIM], ydt, kind="ExternalOutput")

    # Only c_real tokens are real; rows beyond that are padding whose
    # output the host ignores, so the last block shrinks to the real count.
    if c_real is None:
        c_real = C
    blocks = []
    c0 = 0
    while c0 < c_real:
        bn = min(NBLK, c_real - c0)
        blocks.append((c0, bn))
        c0 += bn
    HP2 = HPT // 2  # hidden chunk pairs

    with tile.TileContext(nc) as tc:
        with (
            tc.tile_pool(name="wpool", bufs=1) as wpool,
            tc.tile_pool(name="xpool", bufs=2) as xpool,
            tc.tile_pool(name="hpool", bufs=1) as hpool,
            tc.tile_pool(name="tpool", bufs=2) as tpool,
            tc.tile_pool(name="apool", bufs=2) as apool,
            tc.tile_pool(name="ypool", bufs=3 if YBF16 else 2) as ypool,
            # 4 slots x [P, 1024] f32 = all 8 PSUM banks, one shared tag
            tc.tile_pool(name="psZ", bufs=4, space="PSUM") as psZ,
        ):

            def make_x(bi, c0, bn):
                t = xpool.tile(
                    [P, KD, NBLK], dt.bfloat16, name=f"x_{bi}", tag="x"
                )
                for kd in range(KD):
                    nc.sync.dma_start(t[:, kd, :bn], xt[kd][:, c0 : c0 + bn])
                return t

            def emit_weights():
                # Resident weights, DMA'd in hp-sliced parts in the order the
                # first block's matmuls consume them, split across the SP HWDGE
                # queue (x, w1, w3) and the gpsimd SWDGE queue (w2, wv).
                w1_sb = [
                    wpool.tile([P, HP], dt.bfloat16, name=f"w1_{kd}", tag=f"w1_{kd}")
                    for kd in range(KD)
                ]
                w2_sb = [
                    wpool.tile([P, HP], dt.bfloat16, name=f"w2_{kd}", tag=f"w2_{kd}")
                    for kd in range(KD)
                ]
                w3_sb = [
                    wpool.tile([P, DIM], dt.bfloat16, name=f"w3_{hp}", tag=f"w3_{hp}")
                    for hp in range(HPT)
                ]
                bounds = [0, 3 * P, 7 * P, 12 * P, 17 * P, HP]
                for pi in range(len(bounds) - 1):
                    sl = slice(bounds[pi], bounds[pi + 1])
                    for kd in range(KD):
                        nc.sync.dma_start(w1_sb[kd][:, sl], w1[kd][:, sl])
                    for kd in range(KD):
                        nc.gpsimd.dma_start(w2_sb[kd][:, sl], w2[kd][:, sl])
                for hp in range(HPT):
                    nc.sync.dma_start(w3_sb[hp][:], w3[hp])

                wv_sb = wpool.tile([P, C // P], dt.float32, name="wv_sb", tag="wv_sb")
                nc.gpsimd.dma_start(wv_sb[:], wv[:])
                return w1_sb, w2_sb, w3_sb, wv_sb

            def emit_body(weights, first: bool, x0=None):
                w1_sb, w2_sb, w3_sb, wv_sb = weights
                if x0 is None:
                    # First block's activations, so mm1 can start early.
                    x0 = make_x(0, blocks[0][0], blocks[0][1])

                for bi, (c0, bn) in enumerate(blocks):
                    xb = x0 if bi == 0 else make_x(bi, c0, bn)

                    # hT = silu(w1.T @ xT) * (w2.T @ xT), two hidden chunks
                    # (2*128 rows) per PSUM tile / consumer op.
                    h_sbs = []
                    for hp2 in range(HP2):
                        # NOTE: kd-outer/half-inner interleaving of the two
                        # accumulation groups was tried for rhs reuse and is
                        # NUMERICALLY WRONG on hardware (rel err 4e-2) —
                        # accumulation runs per PSUM region must be contiguous.
                        ps1 = psZ.tile(
                            [P, 2 * bn], dt.float32,
                            name=f"ps1_{bi}_{hp2}", tag="ps",
                        )
                        for half in range(2):
                            hp = 2 * hp2 + half
                            osl = slice(half * bn, (half + 1) * bn)
                            for kd in range(KD):
                                nc.tensor.matmul(
                                    ps1[:, osl],
                                    w1_sb[kd][:, hp * P : (hp + 1) * P],
                                    xb[:, kd, 0:bn],
                                    start=(kd == 0),
                                    stop=(kd == KD - 1),
                                )
                        tsil = tpool.tile(
                            [P, 2 * bn], dt.float32,
                            name=f"sil_{bi}_{hp2}", tag="sil",
                        )
                        nc.scalar.activation(
                            tsil[:], ps1[:], mybir.ActivationFunctionType.Silu
                        )
                        ps2 = psZ.tile(
                            [P, 2 * bn], dt.float32,
                            name=f"ps2_{bi}_{hp2}", tag="ps",
                        )
                        for half in range(2):
                            hp = 2 * hp2 + half
                            osl = slice(half * bn, (half + 1) * bn)
                            for kd in range(KD):
                                nc.tensor.matmul(
                                    ps2[:, osl],
                                    w2_sb[kd][:, hp * P : (hp + 1) * P],
                                    xb[:, kd, 0:bn],
                                    start=(kd == 0),
                                    stop=(kd == KD - 1),
                                )
                        ht = hpool.tile(
                            [P, 2 * bn], dt.bfloat16,
                            name=f"h_{bi}_{hp2}", tag=f"h_{hp2}",
                        )
                        nc.vector.tensor_mul(ht[:], tsil[:], ps2[:])
                        h_sbs.append(ht)

                    # y[block] = (hT.T @ w3) * combine_weight. HW A/B: PSUM
                    # accumulation runs deeper than ~8 into one bank region
                    # degrade the sustained MM rate (~265 ns/MM for 22-deep
                    # vs ~180-200 for 8-deep bursts), so the 22-chunk
                    # contraction is split into 3 partials of depth 8/8/6 —
                    # each a fresh pair of <=8-long bank bursts — and DVE
                    # folds each partial into the output as (p*wv) + acc.
                    for cs in range((bn + P - 1) // P):
                        M = min(P, bn - cs * P)
                        ci = c0 // P + cs
                        acc = None
                        for pi, (lo_hp, hi_hp) in enumerate(
                            [(0, 8), (8, 16), (16, HPT)]
                        ):
                            ps3 = psZ.tile(
                                [P, DIM], dt.float32,
                                name=f"ps3_{bi}_{cs}_{pi}", tag="ps",
                            )
                            for dti in range(DIM // 512):
                                dsl = slice(dti * 512, (dti + 1) * 512)
                                for hp in range(lo_hp, hi_hp):
                                    hp2, half = divmod(hp, 2)
                                    lo = half * bn + cs * P
                                    nc.tensor.matmul(
                                        ps3[:M, dsl],
                                        h_sbs[hp2][:, lo : lo + M],
                                        w3_sb[hp][:, dsl],
                                        start=(hp == lo_hp),
                                        stop=(hp == hi_hp - 1),
                                    )
                            wvs = wv_sb[:M, ci : ci + 1]
                            if pi == 0:
                                a1 = apool.tile(
                                    [P, DIM], dt.float32,
                                    name=f"ya_{bi}_{cs}", tag="ya",
                                )
                                nc.vector.tensor_scalar_mul(a1[:M], ps3[:M], wvs)
                                acc = a1
                            elif pi == 1:
                                a2 = apool.tile(
                                    [P, DIM], dt.float32,
                                    name=f"yb_{bi}_{cs}", tag="yb",
                                )
                                nc.vector.scalar_tensor_tensor(
                                    a2[:M], ps3[:M], wvs, acc[:M],
                                    op0=mybir.AluOpType.mult,
                                    op1=mybir.AluOpType.add,
                                )
                                acc = a2
                            else:
                                yt = ypool.tile(
                                    [P, DIM], ydt, name=f"y_{bi}_{cs}", tag="yt",
                                )
                                nc.vector.scalar_tensor_tensor(
                                    yt[:M], ps3[:M], wvs, acc[:M],
                                    op0=mybir.AluOpType.mult,
                                    op1=mybir.AluOpType.add,
                                )
                        # y goes out on the (otherwise idle) ACT HWDGE queue
                        # so stores don't serialize behind x/w loads
                        nc.scalar.dma_start(
                            y[c0 + cs * P : c0 + cs * P + M, :], yt[:M]
                        )

            if reps > 1:
                # Weights load ONCE, outside the rep loop — matching the
                # single-shot kernel the harness runs (it also loads weights
                # once); the loop then measures pure compute steady-state.
                weights = emit_weights()
                # PE body is ~5.6k instructions (>> one 16 KiB IRAM block):
                # without a branch hint every back-edge stalls ~4 us on the
                # IRAM refetch of the loop-start block.
                with tc.For_i(0, reps, 1, hint_engines=(mybir.EngineType.PE,)):
                    emit_body(weights, first=False)
            else:
                # x0's DMA queues on SP ahead of the weight slices so mm1's
                # first inputs land first.
                x0 = make_x(0, blocks[0][0], blocks[0][1])
                weights = emit_weights()
                emit_body(weights, first=True, x0=x0)

    nc.compile()
    return nc


def _route(xf: np.ndarray, router_w: np.ndarray):
    """Top-2 routing + softmax weights, fp32, matching the jax reference."""
    T = xf.shape[0]
    logits = xf @ router_w  # [T, E]
    rows = np.arange(T)
    i1 = logits.argmax(axis=1)
    tmp = logits.copy()
    tmp[rows, i1] = -np.inf
    i2 = tmp.argmax(axis=1)
    v1 = logits[rows, i1]
    v2 = tmp[rows, i2]
    e2 = np.exp((v2 - v1).astype(np.float32))
    g1 = 1.0 / (1.0 + e2)
    g2 = e2 / (1.0 + e2)
    return i1, i2, g1.astype(np.float32), g2.astype(np.float32)


def _prepare(x, router_w, w1, w2, w3):
    """Route + dispatch on host; returns (C, in_maps, idxs, shape)."""
    x = np.asarray(x, dtype=np.float32)
    router_w = np.asarray(router_w, dtype=np.float32)
    w1 = np.asarray(w1, dtype=np.float32)
    w2 = np.asarray(w2, dtype=np.float32)
    w3 = np.asarray(w3, dtype=np.float32)

    B, S, D = x.shape
    T = B * S
    xf = x.reshape(T, D)

    i1, i2, g1, g2 = _route(xf, router_w)

    # per-expert token lists (slot-0 tokens then slot-1 tokens)
    idxs, wgts = [], []
    for e in range(NUM_EXPERTS):
        s0 = np.nonzero(i1 == e)[0]
        s1 = np.nonzero(i2 == e)[0]
        idxs.append(np.concatenate([s0, s1]))
        wgts.append(np.concatenate([g1[s0], g2[s1]]))
    max_cnt = max(len(ix) for ix in idxs)
    C = max(P, ((max_cnt + P - 1) // P) * P)

    bf16 = ml_dtypes.bfloat16
    # expert weights, padded along the hidden dim and cast to bf16
    w1p = np.zeros((NUM_EXPERTS, D, HP), dtype=bf16)
    w1p[:, :, :HIDDEN] = w1
    w2p = np.zeros((NUM_EXPERTS, D, HP), dtype=bf16)
    w2p[:, :, :HIDDEN] = w2
    w3p = np.zeros((NUM_EXPERTS, HP, D), dtype=bf16)
    w3p[:, :HIDDEN, :] = w3

    in_maps = []
    for e in range(NUM_EXPERTS):
        ix = idxs[e]
        xg = np.zeros((C, D), dtype=np.float32)
        xg[: len(ix)] = xf[ix]
        wvec = np.zeros((C,), dtype=np.float32)
        wvec[: len(ix)] = wgts[e]
        wvec = np.ascontiguousarray(wvec.reshape(C // P, P).T)  # [P, C//P]
        in_maps.append(
            {
                "xt": np.ascontiguousarray(xg.T).astype(bf16).reshape(KD, P, C),
                "w1": w1p[e].reshape(KD, P, HP),
                "w2": w2p[e].reshape(KD, P, HP),
                "w3": w3p[e].reshape(HPT, P, DIM),
                "wv": wvec,
            }
        )
    return C, in_maps, idxs, (B, S, D)


def kernel(x, router_w, w1, w2, w3):
    global LAST_RESULT
    from concourse.bass_utils import run_bass_kernel_spmd

    C, in_maps, idxs, (B, S, D) = _prepare(x, router_w, w1, w2, w3)

    max_cnt = max(len(ix) for ix in idxs)
    key = (C, max_cnt)
    if key not in _KERNELS:
        _KERNELS[key] = _build(C, c_real=max_cnt)
    nc = _KERNELS[key]

    res = run_bass_kernel_spmd(
        nc,
        in_maps,
        list(range(NUM_EXPERTS)),
        trace=TRACE,
    )
    LAST_RESULT = res

    out = np.zeros((B * S, D), dtype=np.float32)
    for e in range(NUM_EXPERTS):
        ix = idxs[e]
        out[ix] += res.results[e]["y"][: len(ix)]
    return out.reshape(B, S, D)



# revision 11
# speedup vs baseline: 1.0247x; 1.0026x over previous
"""MoE layer (8 experts, top-2, SwiGLU) for Trainium2, expert-parallel over 8 cores.

Strategy:
  - Router (x @ router_w, top-2, softmax) runs on host in fp32 — it is 0.01%
    of the FLOPs and determines the (data-dependent) sharding.
  - Each core is assigned one expert. Tokens routed to that expert are
    gathered on host, padded to a common capacity C, and shipped transposed
    as xT [D, C] so both GEMMs need no on-device transpose:
        h1T = w1.T @ x.T   (lhsT = w1 [D,Hp], rhs = xT [D,C])   -> [Hp, C]
        h2T = w2.T @ x.T
        hT  = silu(h1T) * h2T
        y   = hT.T @ w3    (lhsT = hT [Hp,C], rhs = w3 [Hp,D])  -> [C, D]
    y rows are scaled by the per-token combine weight on device.
  - Host scatter-adds the 8 per-expert outputs back to [B,S,D].

  Matmuls run in bf16 (fp32 accumulate in PSUM); hidden dim 2730 is padded
  to 2816 = 22*128 (zero pad is exact: silu(0)*0 = 0).

  HW A/B (microbench.py) showed PSUM bank sequencing dominates PE
  throughput on this silicon: ALL matmul outputs rotate through one PSUM
  pool/tag ("psZ"); accumulation runs per bank region are kept contiguous
  (interleaving two accumulation groups is numerically WRONG on hw), and
  runs deeper than ~8 MMs into one region measure slower (~265 ns/MM for
  22-deep vs ~180-230 for <=8-deep bursts at N=512), so mm3's 22-chunk
  contraction is split into 3 partials of depth 8/8/6 that DVE folds into
  the output as (p*wv)+acc via scalar_tensor_tensor.
  y is stored bf16 (host accumulates in fp32; adds ~2e-4 rel err) and
  leaves on the ACT HWDGE queue to stay off the x/w load queue.

  Measured (rep-slope, interleaved 1/128-rep pairing): ~607-623 us vs
  TimelineSim 504 us — consistent with the P0 power downclock to ~2.0 GHz
  under sustained 8-core matmul load; fp8 (4.6e-2 err) and capacity drops
  (3.7e-2 err) both blow the 2e-2 budget, so bf16 at this schedule is the
  practical wall.
"""

import os

import numpy as np
import ml_dtypes

DIM = 1024
NUM_EXPERTS = 8
HIDDEN = 2730
P = 128
HP = 2816  # hidden padded to 22*128
KD = DIM // P  # 8 contraction chunks for mm1/mm2
HPT = HP // P  # 22 chunks of the hidden dim
NBLK = 512  # token block (moving free dim per matmul)

TRACE = os.environ.get("MOE_TRACE", "0") == "1"
PAIR = os.environ.get("MOE_PAIR", "0") == "1"
YBF16 = os.environ.get("MOE_YBF16", "1") == "1"
LAST_RESULT = None  # BassKernelResults of the last run (for test harness)

_KERNELS: dict = {}


def _build(C: int, c_real: int | None = None, reps: int = 1, pair: bool = PAIR):
    """Build + compile the per-core Bass kernel for capacity C (multiple of 128).

    reps > 1 wraps the compute body (x loads + GEMMs + y stores) in a
    device-side loop that recomputes the same result `reps` times — only
    used for wall-clock benchmarking (dispatch overhead cancels in the rep
    delta). Weights load once, outside the loop, matching the single-shot
    kernel.

    All matmul outputs rotate through ONE 2-bank-tile PSUM pool/tag (HW A/B:
    single-tag rotation beats multi-tag alternation), accumulation runs per
    bank region are contiguous and <=8 deep (see module docstring), and
    consumers are batched two hidden-chunks (mm1/mm2) per instruction to
    halve ACT/DVE PSUM-read op count.
    """
    import concourse.mybir as mybir
    import concourse.tile as tile
    from concourse import bacc

    dt = mybir.dt
    nc = bacc.Bacc(None, target_bir_lowering=False)

    xt = nc.dram_tensor("xt", [KD, P, C], dt.bfloat16, kind="ExternalInput")
    w1 = nc.dram_tensor("w1", [KD, P, HP], dt.bfloat16, kind="ExternalInput")
    w2 = nc.dram_tensor("w2", [KD, P, HP], dt.bfloat16, kind="ExternalInput")
    w3 = nc.dram_tensor("w3", [HPT, P, DIM], dt.bfloat16, kind="ExternalInput")
    wv = nc.dram_tensor("wv", [P, C // P], dt.float32, kind="ExternalInput")
    ydt = dt.bfloat16 if YBF16 else dt.float32
    y = nc.dram_tensor("y", [C, DIM], ydt, kind="ExternalOutput")

    # Only c_real tokens are real; rows beyond that are padding whose
    # output the host ignores, so the last block shrinks to the real count.
    if c_real is None:
        c_real = C
    blocks = []
    c0 = 0
    while c0 < c_real:
        bn = min(NBLK, c_real - c0)
        blocks.append((c0, bn))
        c0 += bn
    HP2 = HPT // 2  # hidden chunk pairs

    with tile.TileContext(nc) as tc:
        with (
            tc.tile_pool(name="wpool", bufs=1) as wpool,
            tc.tile_pool(name="xpool", bufs=2) as xpool,
            tc.tile_pool(name="hpool", bufs=1) as hpool,
            tc.tile_pool(name="tpool", bufs=2) as tpool,
            tc.tile_pool(name="apool", bufs=2) as apool,
            tc.tile_pool(name="ypool", bufs=3 if YBF16 else 2) as ypool,
            # 4 slots x [P, 1024] f32 = all 8 PSUM banks, one shared tag
            tc.tile_pool(name="psZ", bufs=4, space="PSUM") as psZ,
        ):

            def make_x(bi, c0, bn):
                t = xpool.tile(
                    [P, KD, NBLK], dt.bfloat16, name=f"x_{bi}", tag="x"
                )
                for kd in range(KD):
                    nc.sync.dma_start(t[:, kd, :bn], xt[kd][:, c0 : c0 + bn])
                return t

            def emit_weights():
                # Resident weights, DMA'd in hp-sliced parts in the order the
                # first block's matmuls consume them, split across the SP HWDGE
                # queue (x, w1, w3) and the gpsimd SWDGE queue (w2, wv).
                w1_sb = [
                    wpool.tile([P, HP], dt.bfloat16, name=f"w1_{kd}", tag=f"w1_{kd}")
                    for kd in range(KD)
                ]
                w2_sb = [
                    wpool.tile([P, HP], dt.bfloat16, name=f"w2_{kd}", tag=f"w2_{kd}")
                    for kd in range(KD)
                ]
                w3_sb = [
                    wpool.tile([P, DIM], dt.bfloat16, name=f"w3_{hp}", tag=f"w3_{hp}")
                    for hp in range(HPT)
                ]
                bounds = [0, 3 * P, 7 * P, 12 * P, 17 * P, HP]
                for pi in range(len(bounds) - 1):
                    sl = slice(bounds[pi], bounds[pi + 1])
                    for kd in range(KD):
                        nc.sync.dma_start(w1_sb[kd][:, sl], w1[kd][:, sl])
                    for kd in range(KD):
                        nc.gpsimd.dma_start(w2_sb[kd][:, sl], w2[kd][:, sl])
                for hp in range(HPT):
                    nc.sync.dma_start(w3_sb[hp][:], w3[hp])

                wv_sb = wpool.tile([P, C // P], dt.float32, name="wv_sb", tag="wv_sb")
                nc.gpsimd.dma_start(wv_sb[:], wv[:])
                return w1_sb, w2_sb, w3_sb, wv_sb

            def emit_body(weights, first: bool, x0=None):
                w1_sb, w2_sb, w3_sb, wv_sb = weights
                if x0 is None:
                    # First block's activations, so mm1 can start early.
                    x0 = make_x(0, blocks[0][0], blocks[0][1])

                for bi, (c0, bn) in enumerate(blocks):
                    xb = x0 if bi == 0 else make_x(bi, c0, bn)

                    # hT = silu(w1.T @ xT) * (w2.T @ xT), two hidden chunks
                    # (2*128 rows) per PSUM tile / consumer op.
                    h_sbs = []
                    for hp2 in range(HP2):
                        # NOTE: kd-outer/half-inner interleaving of the two
                        # accumulation groups was tried for rhs reuse and is
                        # NUMERICALLY WRONG on hardware (rel err 4e-2) —
                        # accumulation runs per PSUM region must be contiguous.
                        ps1 = psZ.tile(
                            [P, 2 * bn], dt.float32,
                            name=f"ps1_{bi}_{hp2}", tag="ps",
                        )
                        for half in range(2):
                            hp = 2 * hp2 + half
                            osl = slice(half * bn, (half + 1) * bn)
                            for kd in range(KD):
                                nc.tensor.matmul(
                                    ps1[:, osl],
                                    w1_sb[kd][:, hp * P : (hp + 1) * P],
                                    xb[:, kd, 0:bn],
                                    start=(kd == 0),
                                    stop=(kd == KD - 1),
                                )
                        tsil = tpool.tile(
                            [P, 2 * bn], dt.float32,
                            name=f"sil_{bi}_{hp2}", tag="sil",
                        )
                        nc.scalar.activation(
                            tsil[:], ps1[:], mybir.ActivationFunctionType.Silu
                        )
                        ps2 = psZ.tile(
                            [P, 2 * bn], dt.float32,
                            name=f"ps2_{bi}_{hp2}", tag="ps",
                        )
                        for half in range(2):
                            hp = 2 * hp2 + half
                            osl = slice(half * bn, (half + 1) * bn)
                            for kd in range(KD):
                                nc.tensor.matmul(
                                    ps2[:, osl],
                                    w2_sb[kd][:, hp * P : (hp + 1) * P],
                                    xb[:, kd, 0:bn],
                                    start=(kd == 0),
                                    stop=(kd == KD - 1),
                                )
                        ht = hpool.tile(
                            [P, 2 * bn], dt.bfloat16,
                            name=f"h_{bi}_{hp2}", tag=f"h_{hp2}",
                        )
                        nc.vector.tensor_mul(ht[:], tsil[:], ps2[:])
                        h_sbs.append(ht)

                    # y[block] = (hT.T @ w3) * combine_weight. HW A/B: PSUM
                    # accumulation runs deeper than ~8 into one bank region
                    # degrade the sustained MM rate (~265 ns/MM for 22-deep
                    # vs ~180-200 for 8-deep bursts), so the 22-chunk
                    # contraction is split into 3 partials of depth 8/8/6 —
                    # each a fresh pair of <=8-long bank bursts — and DVE
                    # folds each partial into the output as (p*wv) + acc.
                    for cs in range((bn + P - 1) // P):
                        M = min(P, bn - cs * P)
                        ci = c0 // P + cs
                        acc = None
                        for pi, (lo_hp, hi_hp) in enumerate(
                            [(0, 8), (8, 16), (16, HPT)]
                        ):
                            ps3 = psZ.tile(
                                [P, DIM], dt.float32,
                                name=f"ps3_{bi}_{cs}_{pi}", tag="ps",
                            )
                            for dti in range(DIM // 512):
                                dsl = slice(dti * 512, (dti + 1) * 512)
                                for hp in range(lo_hp, hi_hp):
                                    hp2, half = divmod(hp, 2)
                                    lo = half * bn + cs * P
                                    nc.tensor.matmul(
                                        ps3[:M, dsl],
                                        h_sbs[hp2][:, lo : lo + M],
                                        w3_sb[hp][:, dsl],
                                        start=(hp == lo_hp),
                                        stop=(hp == hi_hp - 1),
                                    )
                            wvs = wv_sb[:M, ci : ci + 1]
                            if pi == 0:
                                a1 = apool.tile(
                                    [P, DIM], dt.float32,
                                    name=f"ya_{bi}_{cs}", tag="ya",
                                )
                                nc.vector.tensor_scalar_mul(a1[:M], ps3[:M], wvs)
                                acc = a1
                            elif pi == 1:
                                a2 = apool.tile(
                                    [P, DIM], dt.float32,
                                    name=f"yb_{bi}_{cs}", tag="yb",
                                )
                                nc.vector.scalar_tensor_tensor(
                                    a2[:M], ps3[:M], wvs, acc[:M],
                                    op0=mybir.AluOpType.mult,
                                    op1=mybir.AluOpType.add,
                                )
                                acc = a2
                            else:
                                yt = ypool.tile(
                                    [P, DIM], ydt, name=f"y_{bi}_{cs}", tag="yt",
                                )
                                nc.vector.scalar_tensor_tensor(
                                    yt[:M], ps3[:M], wvs, acc[:M],
                                    op0=mybir.AluOpType.mult,
                                    op1=mybir.AluOpType.add,
                                )
                        # y goes out on the (otherwise idle) ACT HWDGE queue
                        # so stores don't serialize behind x/w loads
                        nc.scalar.dma_start(
                            y[c0 + cs * P : c0 + cs * P + M, :], yt[:M]
                        )

            if reps > 1:
                # Weights load ONCE, outside the rep loop — matching the
                # single-shot kernel the harness runs (it also loads weights
                # once); the loop then measures pure compute steady-state.
                weights = emit_weights()
                # PE body is ~5.6k instructions (>> one 16 KiB IRAM block):
                # without a branch hint every back-edge stalls ~4 us on the
                # IRAM refetch of the loop-start block.
                with tc.For_i(0, reps, 1, hint_engines=(mybir.EngineType.PE,)):
                    emit_body(weights, first=False)
            else:
                # x0's DMA queues on SP ahead of the weight slices so mm1's
                # first inputs land first.
                x0 = make_x(0, blocks[0][0], blocks[0][1])
                weights = emit_weights()
                emit_body(weights, first=True, x0=x0)

    nc.compile()
    return nc


def _route(xf: np.ndarray, router_w: np.ndarray):
    """Top-2 routing + softmax weights, fp32, matching the jax reference."""
    T = xf.shape[0]
    logits = xf @ router_w  # [T, E]
    rows = np.arange(T)
    i1 = logits.argmax(axis=1)
    tmp = logits.copy()
    tmp[rows, i1] = -np.inf
    i2 = tmp.argmax(axis=1)
    v1 = logits[rows, i1]
    v2 = tmp[rows, i2]
    e2 = np.exp((v2 - v1).astype(np.float32))
    g1 = 1.0 / (1.0 + e2)
    g2 = e2 / (1.0 + e2)
    return i1, i2, g1.astype(np.float32), g2.astype(np.float32)


def _prepare(x, router_w, w1, w2, w3):
    """Route + dispatch on host; returns (C, in_maps, idxs, shape)."""
    x = np.asarray(x, dtype=np.float32)
    router_w = np.asarray(router_w, dtype=np.float32)
    w1 = np.asarray(w1, dtype=np.float32)
    w2 = np.asarray(w2, dtype=np.float32)
    w3 = np.asarray(w3, dtype=np.float32)

    B, S, D = x.shape
    T = B * S
    xf = x.reshape(T, D)

    i1, i2, g1, g2 = _route(xf, router_w)

    # per-expert token lists (slot-0 tokens then slot-1 tokens)
    idxs, wgts = [], []
    for e in range(NUM_EXPERTS):
        s0 = np.nonzero(i1 == e)[0]
        s1 = np.nonzero(i2 == e)[0]
        idxs.append(np.concatenate([s0, s1]))
        wgts.append(np.concatenate([g1[s0], g2[s1]]))
    max_cnt = max(len(ix) for ix in idxs)
    C = max(P, ((max_cnt + P - 1) // P) * P)

    bf16 = ml_dtypes.bfloat16
    # expert weights, padded along the hidden dim and cast to bf16
    w1p = np.zeros((NUM_EXPERTS, D, HP), dtype=bf16)
    w1p[:, :, :HIDDEN] = w1
    w2p = np.zeros((NUM_EXPERTS, D, HP), dtype=bf16)
    w2p[:, :, :HIDDEN] = w2
    w3p = np.zeros((NUM_EXPERTS, HP, D), dtype=bf16)
    w3p[:, :HIDDEN, :] = w3

    in_maps = []
    for e in range(NUM_EXPERTS):
        ix = idxs[e]
        xg = np.zeros((C, D), dtype=np.float32)
        xg[: len(ix)] = xf[ix]
        wvec = np.zeros((C,), dtype=np.float32)
        wvec[: len(ix)] = wgts[e]
        wvec = np.ascontiguousarray(wvec.reshape(C // P, P).T)  # [P, C//P]
        in_maps.append(
            {
                "xt": np.ascontiguousarray(xg.T).astype(bf16).reshape(KD, P, C),
                "w1": w1p[e].reshape(KD, P, HP),
                "w2": w2p[e].reshape(KD, P, HP),
                "w3": w3p[e].reshape(HPT, P, DIM),
                "wv": wvec,
            }
        )
    return C, in_maps, idxs, (B, S, D)


def kernel(x, router_w, w1, w2, w3):
    global LAST_RESULT
    from concourse.bass_utils import run_bass_kernel_spmd

    C, in_maps, idxs, (B, S, D) = _prepare(x, router_w, w1, w2, w3)

    max_cnt = max(len(ix) for ix in idxs)
    key = (C, max_cnt)
    if key not in _KERNELS:
        _KERNELS[key] = _build(C, c_real=max_cnt)
    nc = _KERNELS[key]

    res = run_bass_kernel_spmd(
        nc,
        in_maps,
        list(range(NUM_EXPERTS)),
        trace=TRACE,
    )
    LAST_RESULT = res

    out = np.zeros((B * S, D), dtype=np.float32)
    for e in range(NUM_EXPERTS):
        ix = idxs[e]
        out[ix] += res.results[e]["y"][: len(ix)]
    return out.reshape(B, S, D)



# revision 14
# speedup vs baseline: 1.0424x; 1.0173x over previous
"""MoE layer (8 experts, top-2, SwiGLU) for Trainium2, expert-parallel over 8 cores.

Strategy:
  - Router (x @ router_w, top-2, softmax) runs on host in fp32 — it is 0.01%
    of the FLOPs and determines the (data-dependent) sharding.
  - Each core is assigned one expert. Tokens routed to that expert are
    gathered on host, padded to a common capacity C, and shipped transposed
    as xT [D, C] so both GEMMs need no on-device transpose:
        h1T = w1.T @ x.T   (lhsT = w1 [D,Hp], rhs = xT [D,C])   -> [Hp, C]
        h2T = w2.T @ x.T
        hT  = silu(h1T) * h2T
        y   = hT.T @ w3    (lhsT = hT [Hp,C], rhs = w3 [Hp,D])  -> [C, D]
    y rows are scaled by the per-token combine weight on device.
  - Host scatter-adds the 8 per-expert outputs back to [B,S,D].

  Matmuls run in bf16 (fp32 accumulate in PSUM); hidden dim 2730 is padded
  to 2816 = 22*128 (zero pad is exact: silu(0)*0 = 0).

  HW A/B (microbench.py) showed PSUM bank sequencing dominates PE
  throughput on this silicon: ALL matmul outputs rotate through one PSUM
  pool/tag ("psZ"); accumulation runs per bank region are kept contiguous
  (interleaving two accumulation groups is numerically WRONG on hw), and
  runs deeper than ~8 MMs into one region measure slower (~265 ns/MM for
  22-deep vs ~180-230 for <=8-deep bursts at N=512), so mm3's 22-chunk
  contraction is split into 3 partials of depth 8/8/6 that DVE folds into
  the output as (p*wv)+acc via scalar_tensor_tensor.
  y is stored bf16 (host accumulates in fp32; adds ~2e-4 rel err) and
  leaves on the ACT HWDGE queue to stay off the x/w load queue.

  Measured (rep-slope, interleaved 1/128-rep pairing): ~607-623 us vs
  TimelineSim 504 us — consistent with the P0 power downclock to ~2.0 GHz
  under sustained 8-core matmul load; fp8 (4.6e-2 err) and capacity drops
  (3.7e-2 err) both blow the 2e-2 budget, so bf16 at this schedule is the
  practical wall.
"""

import os

import numpy as np
import ml_dtypes

DIM = 1024
NUM_EXPERTS = 8
HIDDEN = 2730
P = 128
HP = 2816  # hidden padded to 22*128
KD = DIM // P  # 8 contraction chunks for mm1/mm2
HPT = HP // P  # 22 chunks of the hidden dim
NBLK = 512  # token block (moving free dim per matmul)

TRACE = os.environ.get("MOE_TRACE", "0") == "1"
PAIR = os.environ.get("MOE_PAIR", "0") == "1"
YBF16 = os.environ.get("MOE_YBF16", "1") == "1"
LAST_RESULT = None  # BassKernelResults of the last run (for test harness)

_KERNELS: dict = {}


def _build(C: int, c_real: int | None = None, reps: int = 1, pair: bool = PAIR):
    """Build + compile the per-core Bass kernel for capacity C (multiple of 128).

    reps > 1 wraps the compute body (x loads + GEMMs + y stores) in a
    device-side loop that recomputes the same result `reps` times — only
    used for wall-clock benchmarking (dispatch overhead cancels in the rep
    delta). Weights load once, outside the loop, matching the single-shot
    kernel.

    All matmul outputs rotate through ONE 2-bank-tile PSUM pool/tag (HW A/B:
    single-tag rotation beats multi-tag alternation), accumulation runs per
    bank region are contiguous and <=8 deep (see module docstring), and
    consumers are batched two hidden-chunks (mm1/mm2) per instruction to
    halve ACT/DVE PSUM-read op count.
    """
    import concourse.mybir as mybir
    import concourse.tile as tile
    from concourse import bacc

    dt = mybir.dt
    nc = bacc.Bacc(None, target_bir_lowering=False)

    xt = nc.dram_tensor("xt", [KD, P, C], dt.bfloat16, kind="ExternalInput")
    w1 = nc.dram_tensor("w1", [KD, P, HP], dt.bfloat16, kind="ExternalInput")
    w2 = nc.dram_tensor("w2", [KD, P, HP], dt.bfloat16, kind="ExternalInput")
    w3 = nc.dram_tensor("w3", [HPT, P, DIM], dt.bfloat16, kind="ExternalInput")
    wv = nc.dram_tensor("wv", [P, C // P], dt.float32, kind="ExternalInput")
    ydt = dt.bfloat16 if YBF16 else dt.float32
    y = nc.dram_tensor("y", [C, DIM], ydt, kind="ExternalOutput")

    # Only c_real tokens are real; rows beyond that are padding whose
    # output the host ignores, so the last block shrinks to the real count.
    if c_real is None:
        c_real = C
    blocks = []
    c0 = 0
    while c0 < c_real:
        bn = min(NBLK, c_real - c0)
        blocks.append((c0, bn))
        c0 += bn
    HP2 = HPT // 2  # hidden chunk pairs

    with tile.TileContext(nc) as tc:
        with (
            tc.tile_pool(name="wpool", bufs=1) as wpool,
            tc.tile_pool(name="xpool", bufs=2) as xpool,
            tc.tile_pool(name="hpool", bufs=1) as hpool,
            tc.tile_pool(name="tpool", bufs=2) as tpool,
            tc.tile_pool(name="apool", bufs=2) as apool,
            tc.tile_pool(name="ypool", bufs=3 if YBF16 else 2) as ypool,
            # 8 slots x [P, 512] f32 = all 8 PSUM banks, one shared tag
            tc.tile_pool(name="psZ", bufs=8, space="PSUM") as psZ,
        ):

            def make_x(bi, c0, bn):
                t = xpool.tile(
                    [P, KD, NBLK], dt.bfloat16, name=f"x_{bi}", tag="x"
                )
                for kd in range(KD):
                    nc.sync.dma_start(t[:, kd, :bn], xt[kd][:, c0 : c0 + bn])
                return t

            def emit_weights():
                # Resident weights, DMA'd in hp-sliced parts in the order the
                # first block's matmuls consume them, split across the SP HWDGE
                # queue (x, w1, w3) and the gpsimd SWDGE queue (w2, wv).
                w1_sb = [
                    wpool.tile([P, HP], dt.bfloat16, name=f"w1_{kd}", tag=f"w1_{kd}")
                    for kd in range(KD)
                ]
                w2_sb = [
                    wpool.tile([P, HP], dt.bfloat16, name=f"w2_{kd}", tag=f"w2_{kd}")
                    for kd in range(KD)
                ]
                w3_sb = [
                    wpool.tile([P, DIM], dt.bfloat16, name=f"w3_{hp}", tag=f"w3_{hp}")
                    for hp in range(HPT)
                ]
                bounds = [0, 3 * P, 7 * P, 12 * P, 17 * P, HP]
                for pi in range(len(bounds) - 1):
                    sl = slice(bounds[pi], bounds[pi + 1])
                    for kd in range(KD):
                        nc.sync.dma_start(w1_sb[kd][:, sl], w1[kd][:, sl])
                    for kd in range(KD):
                        nc.gpsimd.dma_start(w2_sb[kd][:, sl], w2[kd][:, sl])
                for hp in range(HPT):
                    nc.sync.dma_start(w3_sb[hp][:], w3[hp])

                wv_sb = wpool.tile([P, C // P], dt.float32, name="wv_sb", tag="wv_sb")
                nc.gpsimd.dma_start(wv_sb[:], wv[:])
                return w1_sb, w2_sb, w3_sb, wv_sb

            def emit_body(weights, first: bool, x0=None):
                w1_sb, w2_sb, w3_sb, wv_sb = weights
                if x0 is None:
                    # First block's activations, so mm1 can start early.
                    x0 = make_x(0, blocks[0][0], blocks[0][1])

                for bi, (c0, bn) in enumerate(blocks):
                    xb = x0 if bi == 0 else make_x(bi, c0, bn)

                    # hT = silu(w1.T @ xT) * (w2.T @ xT), one hidden chunk
                    # (128 rows) per single-bank PSUM tile / consumer op.
                    # NOTE: kd-outer interleaving of accumulation groups was
                    # tried for rhs reuse and is NUMERICALLY WRONG on hardware
                    # (rel err 4e-2) — accumulation runs per PSUM region must
                    # be contiguous.
                    h_sbs = []
                    for hp in range(HPT):
                        ps1 = psZ.tile(
                            [P, bn], dt.float32,
                            name=f"ps1_{bi}_{hp}", tag="ps",
                        )
                        for kd in range(KD):
                            nc.tensor.matmul(
                                ps1[:],
                                w1_sb[kd][:, hp * P : (hp + 1) * P],
                                xb[:, kd, 0:bn],
                                start=(kd == 0),
                                stop=(kd == KD - 1),
                            )
                        tsil = tpool.tile(
                            [P, bn], dt.float32,
                            name=f"sil_{bi}_{hp}", tag="sil",
                        )
                        nc.scalar.activation(
                            tsil[:], ps1[:], mybir.ActivationFunctionType.Silu
                        )
                        ps2 = psZ.tile(
                            [P, bn], dt.float32,
                            name=f"ps2_{bi}_{hp}", tag="ps",
                        )
                        for kd in range(KD):
                            nc.tensor.matmul(
                                ps2[:],
                                w2_sb[kd][:, hp * P : (hp + 1) * P],
                                xb[:, kd, 0:bn],
                                start=(kd == 0),
                                stop=(kd == KD - 1),
                            )
                        ht = hpool.tile(
                            [P, bn], dt.bfloat16,
                            name=f"h_{bi}_{hp}", tag=f"h_{hp}",
                        )
                        nc.vector.tensor_mul(ht[:], tsil[:], ps2[:])
                        h_sbs.append(ht)

                    # y[block] = (hT.T @ w3) * combine_weight. HW A/B: PSUM
                    # accumulation runs deeper than ~8 into one bank region
                    # degrade the sustained MM rate (~265 ns/MM for 22-deep
                    # vs ~180-200 for 8-deep bursts), so the 22-chunk
                    # contraction is split into 3 partials of depth 8/8/6 —
                    # each a fresh pair of <=8-long bank bursts — and DVE
                    # folds each partial into the output as (p*wv) + acc.
                    for cs in range((bn + P - 1) // P):
                        M = min(P, bn - cs * P)
                        ci = c0 // P + cs
                        wvs = wv_sb[:M, ci : ci + 1]
                        a1 = apool.tile(
                            [P, DIM], dt.float32, name=f"ya_{bi}_{cs}", tag="ya",
                        )
                        a2 = apool.tile(
                            [P, DIM], dt.float32, name=f"yb_{bi}_{cs}", tag="yb",
                        )
                        yt = ypool.tile(
                            [P, DIM], ydt, name=f"y_{bi}_{cs}", tag="yt",
                        )
                        for pi, (lo_hp, hi_hp) in enumerate(
                            [(0, 8), (8, 16), (16, HPT)]
                        ):
                            for dti in range(DIM // 512):
                                dsl = slice(dti * 512, (dti + 1) * 512)
                                ps3 = psZ.tile(
                                    [P, 512], dt.float32,
                                    name=f"ps3_{bi}_{cs}_{pi}_{dti}", tag="ps",
                                )
                                for hp in range(lo_hp, hi_hp):
                                    nc.tensor.matmul(
                                        ps3[:M, :],
                                        h_sbs[hp][:, cs * P : cs * P + M],
                                        w3_sb[hp][:, dsl],
                                        start=(hp == lo_hp),
                                        stop=(hp == hi_hp - 1),
                                    )
                                if pi == 0:
                                    nc.vector.tensor_scalar_mul(
                                        a1[:M, dsl], ps3[:M, :], wvs
                                    )
                                elif pi == 1:
                                    nc.vector.scalar_tensor_tensor(
                                        a2[:M, dsl], ps3[:M, :], wvs, a1[:M, dsl],
                                        op0=mybir.AluOpType.mult,
                                        op1=mybir.AluOpType.add,
                                    )
                                else:
                                    nc.vector.scalar_tensor_tensor(
                                        yt[:M, dsl], ps3[:M, :], wvs, a2[:M, dsl],
                                        op0=mybir.AluOpType.mult,
                                        op1=mybir.AluOpType.add,
                                    )
                        # y goes out on the (otherwise idle) ACT HWDGE queue
                        # so stores don't serialize behind x/w loads
                        nc.scalar.dma_start(
                            y[c0 + cs * P : c0 + cs * P + M, :], yt[:M]
                        )

            if reps > 1:
                # Weights load ONCE, outside the rep loop — matching the
                # single-shot kernel the harness runs (it also loads weights
                # once); the loop then measures pure compute steady-state.
                weights = emit_weights()
                # PE body is ~5.6k instructions (>> one 16 KiB IRAM block):
                # without a branch hint every back-edge stalls ~4 us on the
                # IRAM refetch of the loop-start block.
                with tc.For_i(0, reps, 1, hint_engines=(mybir.EngineType.PE,)):
                    emit_body(weights, first=False)
            else:
                # x0's DMA queues on SP ahead of the weight slices so mm1's
                # first inputs land first.
                x0 = make_x(0, blocks[0][0], blocks[0][1])
                weights = emit_weights()
                emit_body(weights, first=True, x0=x0)

    nc.compile()
    return nc


def _route(xf: np.ndarray, router_w: np.ndarray):
    """Top-2 routing + softmax weights, fp32, matching the jax reference."""
    T = xf.shape[0]
    logits = xf @ router_w  # [T, E]
    rows = np.arange(T)
    i1 = logits.argmax(axis=1)
    tmp = logits.copy()
    tmp[rows, i1] = -np.inf
    i2 = tmp.argmax(axis=1)
    v1 = logits[rows, i1]
    v2 = tmp[rows, i2]
    e2 = np.exp((v2 - v1).astype(np.float32))
    g1 = 1.0 / (1.0 + e2)
    g2 = e2 / (1.0 + e2)
    return i1, i2, g1.astype(np.float32), g2.astype(np.float32)


def _prepare(x, router_w, w1, w2, w3):
    """Route + dispatch on host; returns (C, in_maps, idxs, shape)."""
    x = np.asarray(x, dtype=np.float32)
    router_w = np.asarray(router_w, dtype=np.float32)
    w1 = np.asarray(w1, dtype=np.float32)
    w2 = np.asarray(w2, dtype=np.float32)
    w3 = np.asarray(w3, dtype=np.float32)

    B, S, D = x.shape
    T = B * S
    xf = x.reshape(T, D)

    i1, i2, g1, g2 = _route(xf, router_w)

    # per-expert token lists (slot-0 tokens then slot-1 tokens)
    idxs, wgts = [], []
    for e in range(NUM_EXPERTS):
        s0 = np.nonzero(i1 == e)[0]
        s1 = np.nonzero(i2 == e)[0]
        idxs.append(np.concatenate([s0, s1]))
        wgts.append(np.concatenate([g1[s0], g2[s1]]))
    max_cnt = max(len(ix) for ix in idxs)
    C = max(P, ((max_cnt + P - 1) // P) * P)

    bf16 = ml_dtypes.bfloat16
    # expert weights, padded along the hidden dim and cast to bf16
    w1p = np.zeros((NUM_EXPERTS, D, HP), dtype=bf16)
    w1p[:, :, :HIDDEN] = w1
    w2p = np.zeros((NUM_EXPERTS, D, HP), dtype=bf16)
    w2p[:, :, :HIDDEN] = w2
    w3p = np.zeros((NUM_EXPERTS, HP, D), dtype=bf16)
    w3p[:, :HIDDEN, :] = w3

    in_maps = []
    for e in range(NUM_EXPERTS):
        ix = idxs[e]
        xg = np.zeros((C, D), dtype=np.float32)
        xg[: len(ix)] = xf[ix]
        wvec = np.zeros((C,), dtype=np.float32)
        wvec[: len(ix)] = wgts[e]
        wvec = np.ascontiguousarray(wvec.reshape(C // P, P).T)  # [P, C//P]
        in_maps.append(
            {
                "xt": np.ascontiguousarray(xg.T).astype(bf16).reshape(KD, P, C),
                "w1": w1p[e].reshape(KD, P, HP),
                "w2": w2p[e].reshape(KD, P, HP),
                "w3": w3p[e].reshape(HPT, P, DIM),
                "wv": wvec,
            }
        )
    return C, in_maps, idxs, (B, S, D)


def kernel(x, router_w, w1, w2, w3):
    global LAST_RESULT
    from concourse.bass_utils import run_bass_kernel_spmd

    C, in_maps, idxs, (B, S, D) = _prepare(x, router_w, w1, w2, w3)

    max_cnt = max(len(ix) for ix in idxs)
    key = (C, max_cnt)
    if key not in _KERNELS:
        _KERNELS[key] = _build(C, c_real=max_cnt)
    nc = _KERNELS[key]

    res = run_bass_kernel_spmd(
        nc,
        in_maps,
        list(range(NUM_EXPERTS)),
        trace=TRACE,
    )
    LAST_RESULT = res

    out = np.zeros((B * S, D), dtype=np.float32)
    for e in range(NUM_EXPERTS):
        ix = idxs[e]
        out[ix] += res.results[e]["y"][: len(ix)]
    return out.reshape(B, S, D)

